# revision 7
# baseline (speedup 1.0000x reference)
"""SAN aggregation kernel for Trainium2 (Bass/Tile), 8-core data-parallel.

Problem: out[n,c,h,w] = sum_k w[n, c//8, k, h*W+w] * xpad[n, c, h+dh(k), w+dw(k)]
  x: [8, 64, 128, 128] f32, w: [8, 8, 9, 16384] f32, 3x3 window, pad 1.

Sharding: batch dim N=8 across 8 NeuronCores (1 image per core, no
cross-core communication).

Per-core layout (everything resident in SBUF):
  partitions p = hb*8 + cw   (hb: 16 row-blocks of 8 rows, cw: 8 weight chans)
  x_sb  [128, 8*10*130]: per (gl): rows [hb*8-1, hb*8+9) of channel
        c=cw*8+gl, each row stored with 130-px pitch (1 left + 128 + 1 right
        zero pad).  Shift (dh,dw) == flat offset dh*130+dw.
  w_sb  [128, 9*1024]:  w[cw, k, hb-rows] per partition, k-major.
  out_sb[128, 8*1024]:  out channels (cw,gl) at rows hb.
"""

import sys
import os

for _p in ("/opt/trn_rl_repo", "/root/.axon_site/_ro/trn_rl_repo"):
    if _p not in sys.path and os.path.isdir(_p):
        sys.path.append(_p)

import numpy as np

import concourse.bass as bass
import concourse.bacc as bacc
import concourse.mybir as mybir
import bass_rust
from concourse.tile import TileContext

F32 = mybir.dt.float32

C, H, W = 64, 128, 128
S = H * W          # 16384
CW, GL = 8, 8      # weight channels, share planes
HB = 16            # row blocks
RB = H // HB       # rows per block = 8
PITCH = W + 2      # 130
XROWS = RB + 2     # 10 rows incl halo
XGL = XROWS * PITCH  # 1300 elements per gl block in x_sb
SB = RB * W        # 1024 spatial elems per partition per gl


def _ap(base, dims, extra_offset=0):
    """Copy AP `base`, replace its [step,count] dims, bump offset.

    For SBUF APs dims[0] is the partition dim: pass step "P" to substitute
    the base AP's own partition stride (flat element space, = free width).
    """
    c = base.copy()
    pstep = base.ap[0][0]
    dims = [[pstep if s == "P" else s, n] for s, n in dims]
    c.ap = bass_rust.VecI64Pair(dims)
    if extra_offset:
        c.offset = c.offset + extra_offset
    return c


def build_program():
    nc = bacc.Bacc("TRN2", target_bir_lowering=False, debug=False)
    x_d = nc.dram_tensor("x", [C, S], F32, kind="ExternalInput")
    w_d = nc.dram_tensor("w", [CW, 9, S], F32, kind="ExternalInput")
    o_d = nc.dram_tensor("out", [C, S], F32, kind="ExternalOutput")

    with TileContext(nc) as tc:
        with tc.tile_pool(name="main", bufs=1) as pool, \
             tc.tile_pool(name="tmps", bufs=2) as tpool:
            x_sb = pool.tile([128, GL * XGL], F32)
            w_sb = pool.tile([128, 9 * SB], F32)
            o_sb = pool.tile([128, GL * SB], F32)

            # ---- zero the padding of x_sb (left/right cols, halo rows).
            # Halo rows are zeroed on ALL partitions; the r=0 / r=9 DMAs
            # below overwrite the valid parts (Tile orders WAW deps).
            nc.gpsimd.memset(
                _ap(x_sb[:], [["P", 128], [XGL, GL], [PITCH, RB],
                              [PITCH - 1, 2]], extra_offset=PITCH), 0.0)
            nc.gpsimd.memset(
                _ap(x_sb[:], [["P", 128], [XGL, GL], [1, PITCH]]), 0.0)
            nc.gpsimd.memset(
                _ap(x_sb[:], [["P", 128], [XGL, GL], [1, PITCH]],
                    extra_offset=(XROWS - 1) * PITCH), 0.0)

            # ---- load x: one DMA per (gl, r).  Row r holds x row
            # h = hb*8 + r - 1 of channel cw*8+gl; r=0 invalid at hb=0
            # (memset) and r=9 invalid at hb=15 (memset).
            for gl in range(GL):
                for r in range(XROWS):
                    if r == 0:
                        dst, nhb, src_off = x_sb[8:128], HB - 1, (RB - 1) * W
                    elif r == XROWS - 1:
                        dst, nhb, src_off = x_sb[0:120], HB - 1, RB * W
                    else:
                        dst, nhb, src_off = x_sb[:], HB, (r - 1) * W
                    nc.sync.dma_start(
                        out=_ap(dst, [["P", nhb * CW], [1, W]],
                                extra_offset=gl * XGL + r * PITCH + 1),
                        in_=_ap(x_d.ap(), [[RB * W, nhb], [GL * S, CW], [1, W]],
                                extra_offset=gl * S + src_off))

            # ---- load w ----
            nc.sync.dma_start(
                out=_ap(w_sb[:], [["P", 128], [SB, 9], [1, SB]]),
                in_=_ap(w_d.ap(), [[SB, HB], [9 * S, CW], [S, 9], [1, SB]]))

            # ---- compute ----
            for gl in range(GL):
                acc = _ap(o_sb[:], [["P", 128], [W, RB], [1, W]],
                          extra_offset=gl * SB)
                for k in range(9):
                    dh, dw = divmod(k, 3)
                    xv = _ap(x_sb[:], [["P", 128], [PITCH, RB], [1, W]],
                             extra_offset=gl * XGL + dh * PITCH + dw)
                    wv = _ap(w_sb[:], [["P", 128], [W, RB], [1, W]],
                             extra_offset=k * SB)
                    if k == 0:
                        nc.vector.tensor_mul(out=acc, in0=xv, in1=wv)
                    else:
                        tmp = tpool.tile([128, SB], F32, tag="tmp")
                        t = _ap(tmp[:], [["P", 128], [W, RB], [1, W]])
                        nc.vector.tensor_mul(out=t, in0=xv, in1=wv)
                        nc.vector.tensor_add(out=acc, in0=acc, in1=t)

            # ---- store ----
            nc.sync.dma_start(
                out=_ap(o_d.ap(), [[RB * W, HB], [GL * S, CW], [S, GL],
                                   [1, SB]]),
                in_=_ap(o_sb[:], [["P", 128], [SB, GL], [1, SB]]))

    nc.compile()
    return nc


_NC_CACHE = None


def _get_nc():
    global _NC_CACHE
    if _NC_CACHE is None:
        _NC_CACHE = build_program()
    return _NC_CACHE


def kernel(input, weight):
    """input: [8,64,128,128] f32, weight: [8,8,9,16384] f32 ->
    [8,64,128,128] f32."""
    from concourse.bass_utils import run_bass_kernel_spmd

    x = np.ascontiguousarray(np.asarray(input, dtype=np.float32))
    w = np.ascontiguousarray(np.asarray(weight, dtype=np.float32))
    N = x.shape[0]
    nc = _get_nc()
    in_maps = [{"x": x[i].reshape(C, S), "w": w[i].reshape(CW, 9, S)}
               for i in range(N)]
    res = run_bass_kernel_spmd(nc, in_maps, core_ids=list(range(N)))
    out = np.stack([res.results[i]["out"].reshape(C, H, W) for i in range(N)])
    return out
